# revision 1
# baseline (speedup 1.0000x reference)
"""Trainium2 Bass kernel for nn_BlastLinear (block low-rank linear layer).

Math (reference):
  y[q,n,r] = sum_c x[n, q*1024+c] * C[q,r,c]          (mm1, per input block q)
  z[p,n,r] = sum_q D[p,q,r] * y[q,n,r]                (tiny mix over q)
  o[p,n,j] = sum_r z[p,n,r] * B[p,j,r]                (mm2, per output block p)
  out[n, p*1024+j] = o[p,n,j] + bias[p*1024+j]

Sharding: pure data-parallel over the 8192 tokens -> 1024 tokens per core,
weights replicated, no collectives.

Precision: single-pass bf16 matmuls (PSUM accumulates fp32). The harness
gate is rel_err < 2e-2; bf16 rounding of x/C/B plus the bf16 y/z/out
carries measures ~4e-3 relative error - comfortably in. This is 1 PE
pass per matmul vs 3 for the f32r-split scheme (the previous 366us
version), putting PE at the 512-matmul floor:
512 matmuls x 512 cols x 0.4167 ns = 109.2 us; measured 123.9 us
(PE 88% busy; rest is DMA cold-start, ramp and drain tails).

Per-core structure (chunk = 512 tokens, 2 chunks, PE order
mm1(c0), mm1(c1), mm2(c0), mm2(c1) so mm2 never waits on the mix):
  mm1:  psum y[q,rt] [128r x 512n] += ct^T @ xt   (PE, 8 k-tiles per q)
  ycp:  yb = bf16(y)  PSUM->SBUF on ACT           (keeps DVE off PSUM)
  mix:  zb[p,rt] += D[p,q,rt]*yb[q,rt] as mul(DVE 4x bf16) +
        add(DVE 2x bf16, q2 adds on Pool)         (stt is DVE-only on HW)
  mm2:  psum oT[g] [128o x 512n] += bt^T @ zb     (PE, 4 rt-tiles)
  drain: Identity-activation with per-partition bias AP fuses the bias
        add into the PSUM drain, alternating ACT/DVE so consecutive
        o-groups drain in parallel (2 PSUM o-banks); out ships bf16
        TRANSPOSED [OUT_F, n_core]; host un-transposes + casts to f32.
DMA notes: per-DMA descriptor-gen is ~625ns on one shared HWDGE device,
so transfers are batched (x in 4-k-tile batches, out in 8-group
batches) - ~50 DMAs total, ~24 MiB, hidden under PE except the
cold start. C (4 MiB) + B (4 MiB) bf16 stay resident in SBUF; B
prefetches in 512KiB pieces during chunk 1's mm1; the kernel's final
out-groups ship as small split DMAs to shorten the end tail.
"""

import numpy as np

import concourse.mybir as mybir
import concourse.tile as tile
from concourse import bacc
from concourse.bass_utils import run_bass_kernel_spmd

N_CORES = 8
IN_F = 4096
OUT_F = 4096
P = 4
Q = 4
R = 512
CB = IN_F // Q        # 1024 input features per q block
OB = OUT_F // P       # 1024 output features per p block
N_TOK = 4 * 2048      # 8192 total tokens
N_CORE = N_TOK // N_CORES   # 1024 tokens per core

CHUNK = 512           # tokens per pipeline chunk
KT1 = CB // 128       # 8 contraction tiles per q in mm1
RT = R // 128         # 4 rank partition tiles
KB = 4                # k-tiles per x DMA batch
OG = 8                # o-groups per out DMA batch
OT = OB // 128        # 8 output-feature tiles per p

F32 = mybir.dt.float32
BF16 = mybir.dt.bfloat16
ADD = mybir.AluOpType.add
IDENT = mybir.ActivationFunctionType.Identity

_cached_nc = None


def _build(n_core=N_CORE, chunk=CHUNK):
    nc = bacc.Bacc("TRN2", target_bir_lowering=False, debug=False,
                   enable_asserts=False)

    def din(name, shape, dtype):
        return nc.dram_tensor(name, shape, dtype, kind="ExternalInput").ap()

    xt = din("xt", [IN_F, n_core], BF16)
    ct = din("ct", [IN_F, R], BF16)
    bt = din("bt", [P * R, OB], BF16)
    dr = din("dr", [R, P * Q], F32)
    biasd = din("biasd", [OUT_F], F32)
    outT = nc.dram_tensor("outT", [OUT_F, n_core], BF16,
                          kind="ExternalOutput").ap()

    n_chunks = n_core // chunk

    with tile.TileContext(nc) as tc:
        with (
            tc.tile_pool(name="const", bufs=1) as cpool,
            tc.tile_pool(name="xp", bufs=4) as xpool,
            tc.tile_pool(name="ybp", bufs=16) as ybpool,
            tc.tile_pool(name="tp", bufs=6) as tpool,
            tc.tile_pool(name="zbp", bufs=2 * P * RT * n_chunks) as zbpool,
            tc.tile_pool(name="outp", bufs=4) as outpool,
            tc.tile_pool(name="yps", bufs=6, space="PSUM") as ypool,
            tc.tile_pool(name="ops", bufs=2, space="PSUM") as opool,
        ):
            # ct_sb[c_, q*8+k, r]: C^T tile rows c = (q*8+k)*128 + c_
            ct_sb = cpool.tile([128, IN_F // 128, R], BF16)
            ct3 = ct.rearrange("(t p) r -> p t r", p=128)
            # bt_sb[r_, p*4+rt, o]: B^T tile rows r = (p*4+rt)*128 + r_
            bt_sb = cpool.tile([128, (P * R) // 128, OB], BF16)
            bt3 = bt.rearrange("(t p) o -> p t o", p=128)
            # d_sb[r_, rt, p*4+q] = D[p, q, rt*128 + r_]
            d_sb = cpool.tile([128, RT, P * Q], F32)
            # bias_sb[o_, g] = bias[g*128 + o_]  (g = p*OT + ot)
            bias_sb = cpool.tile([128, OUT_F // 128], F32)

            zb = {}

            def emit_mm1(j):
                for q in range(Q):
                    ys = [
                        ypool.tile([128, chunk], F32, tag="y",
                                   name=f"y_{j}_{q}_{rt}")
                        for rt in range(RT)
                    ]
                    for kb in range(KT1 // KB):
                        base_t = q * KT1 + kb * KB
                        first = j == 0 and q == 0 and kb == 0
                        x_t = xpool.tile([128, KB, chunk], BF16, tag="x",
                                         name=f"x_{j}_{q}_{kb}")

                        def xdma(lo, hi):
                            nc.sync.dma_start(
                                x_t[:, lo:hi, :],
                                xt[(base_t + lo) * 128:(base_t + hi) * 128,
                                   j * chunk:(j + 1) * chunk]
                                .rearrange("(t p) n -> p t n", p=128))

                        def cdma(lo, hi):
                            hs = slice(base_t + lo, base_t + hi)
                            nc.sync.dma_start(ct_sb[:, hs, :], ct3[:, hs, :])

                        if first:
                            # k0's C then x ship alone so the first matmul
                            # waits on ~256 KiB, then interleaved k1 / k2-3
                            # pieces so compute overlaps the cold DMA stream
                            cdma(0, 1)
                            xdma(0, 1)
                            cdma(1, 2)
                            xdma(1, 2)
                            cdma(2, KB)
                            xdma(2, KB)
                        elif j == 0 and q < 2:
                            # half-batches while the DMA pipeline fills:
                            # delivery order matches PE consumption order
                            cdma(0, 2)
                            xdma(0, 2)
                            cdma(2, KB)
                            xdma(2, KB)
                        else:
                            if j == 0:
                                cdma(0, KB)
                            xdma(0, KB)
                        if j == 0 and q == 0 and kb == 1:
                            nc.sync.dma_start(
                                d_sb[:],
                                dr.rearrange("(t p) s -> p t s", p=128))
                            nc.sync.dma_start(
                                bias_sb[:],
                                biasd.rearrange("(t p) -> p t", p=128))
                        if j == n_chunks - 1:
                            # B prefetch in 512KiB pieces spread over the
                            # last mm1 chunk (8 slots): small enough not to
                            # starve the x stream, early enough for mm2 c0
                            idx = q * 2 + kb
                            bs = slice(idx * 2, idx * 2 + 2)
                            nc.sync.dma_start(bt_sb[:, bs, :], bt3[:, bs, :])
                        for kk in range(KB):
                            k = kb * KB + kk
                            for rt in range(RT):
                                nc.tensor.matmul(
                                    ys[rt][:],
                                    lhsT=ct_sb[:, base_t + kk,
                                               rt * 128:(rt + 1) * 128],
                                    rhs=x_t[:, kk, :],
                                    start=(k == 0), stop=(k == KT1 - 1))
                    # y -> SBUF bf16 on ACT, then the D-mix on DVE+Pool
                    ybs = []
                    for rt in range(RT):
                        yb_t = ybpool.tile([128, chunk], BF16, tag="yb",
                                           name=f"yb_{j}_{q}_{rt}")
                        nc.scalar.copy(yb_t[:], ys[rt][:])
                        ybs.append(yb_t)
                    # TensorScalarPtr is DVE-only on real HW (walrus rejects
                    # it on Pool), but TensorTensor runs on Pool too. bf16
                    # SBUF ops hit the DVE fast paths: tensor_scalar_mul at
                    # 4x, tensor_tensor add at 2x -- mul+add (520ns) beats
                    # the fused 1x stt (593ns). The q2 adds go to the
                    # otherwise-idle Pool engine so DVE stays under mm1's
                    # per-q cadence and its queue never backs up into mm2.
                    for rt in range(RT):
                        for p in range(P):
                            col = p * Q + q
                            dcol = d_sb[:, rt, col:col + 1]
                            if q == 0:
                                zt = zbpool.tile([128, chunk], BF16, tag="zb",
                                                 name=f"zb_{j}_{p}_{rt}")
                                zb[(j, p, rt)] = zt
                                nc.vector.tensor_scalar_mul(
                                    zt[:], ybs[rt][:], dcol)
                            else:
                                zt = zb[(j, p, rt)]
                                tt = tpool.tile([128, chunk], BF16, tag="t",
                                                name=f"t_{j}_{q}_{p}_{rt}")
                                nc.vector.tensor_scalar_mul(
                                    tt[:], ybs[rt][:], dcol)
                                eng = nc.gpsimd if q == 2 else nc.vector
                                eng.tensor_tensor(
                                    zt[:], tt[:], zt[:], op=ADD)

            def emit_mm2(j):
                ob_t = None
                for p in range(P):
                    for ot in range(OT):
                        g = p * OT + ot
                        ops = opool.tile([128, chunk], F32, tag="o",
                                         name=f"o_{j}_{g}")
                        for rt in range(RT):
                            nc.tensor.matmul(
                                ops[:],
                                lhsT=bt_sb[:, p * RT + rt,
                                           ot * 128:(ot + 1) * 128],
                                rhs=zb[(j, p, rt)][:],
                                start=(rt == 0), stop=(rt == RT - 1))
                        # batch OG o-groups per out DMA: per-DMA HWDGE
                        # descriptor-gen is a fixed ~625ns on a single
                        # shared device, so per-group DMAs can't keep up
                        # with the 873ns group cadence. The kernel's last
                        # few groups ship in smaller pieces so the final
                        # DMA is small and starts right after its drain.
                        NG = P * OT
                        last = j == n_chunks - 1
                        og = OG
                        if g % og == 0:
                            ob_t = outpool.tile([128, og, chunk], BF16,
                                                tag="ob", name=f"ob_{j}_{g}")
                        dst = ob_t[:, g % og, :]
                        if last and g >= NG - 2:
                            # final two groups: drain column-halves on ACT
                            # and DVE in parallel, each half shipped as its
                            # own 128KiB DMA -- shortens the end-of-kernel
                            # drain+DMA tail
                            h = chunk // 2
                            nc.scalar.activation(
                                dst[:, 0:h], ops[:, 0:h], IDENT,
                                bias=bias_sb[:, g:g + 1], scale=1.0)
                            nc.vector.tensor_scalar_add(
                                dst[:, h:chunk], ops[:, h:chunk],
                                bias_sb[:, g:g + 1])
                            nc.sync.dma_start(
                                outT[g * 128:(g + 1) * 128,
                                     j * chunk:j * chunk + h],
                                dst[:, 0:h])
                            nc.sync.dma_start(
                                outT[g * 128:(g + 1) * 128,
                                     j * chunk + h:(j + 1) * chunk],
                                dst[:, h:chunk])
                            continue
                        # fused PSUM drain + per-partition bias add.
                        # Alternate ACT/DVE so consecutive o-groups drain in
                        # parallel: with only 2 PSUM o-banks, a single
                        # engine's ~950ns drain latency exceeds the 873ns
                        # group time and stalls PE ~120ns/group. In mm2 c0
                        # DVE is still draining chunk 1's mix queue, so all
                        # of c0 stays on ACT.
                        use_dve = (g % 2 == 1) and last
                        if use_dve:
                            nc.vector.tensor_scalar_add(
                                dst, ops[:], bias_sb[:, g:g + 1])
                        else:
                            nc.scalar.activation(
                                dst, ops[:], IDENT,
                                bias=bias_sb[:, g:g + 1], scale=1.0)
                        if last and NG - 4 <= g < NG - 2:
                            # groups 28/29: per-group DMAs
                            nc.sync.dma_start(
                                outT[g * 128:(g + 1) * 128,
                                     j * chunk:(j + 1) * chunk],
                                dst)
                        elif g % og == og - 1 or (last and g == NG - 5):
                            # flush accumulated slots (a full batch, or the
                            # partial batch ending right before the tail)
                            nslot = g % og + 1
                            gb = g - (g % og)
                            nc.sync.dma_start(
                                outT[gb * 128:(g + 1) * 128,
                                     j * chunk:(j + 1) * chunk]
                                .rearrange("(t p) n -> p t n", p=128),
                                ob_t[:, 0:nslot, :])

            for j in range(n_chunks):
                emit_mm1(j)
            for j in range(n_chunks):
                emit_mm2(j)

    nc.compile()
    return nc


def _prep_in_maps(x, B, C, D, bias):
    import ml_dtypes
    x2 = np.asarray(x, dtype=np.float32).reshape(N_TOK, IN_F)
    CT = np.ascontiguousarray(
        np.asarray(C, dtype=np.float32).transpose(0, 2, 1).reshape(IN_F, R)
    ).astype(ml_dtypes.bfloat16)
    BT = np.ascontiguousarray(
        np.asarray(B, dtype=np.float32).transpose(0, 2, 1).reshape(P * R, OB)
    ).astype(ml_dtypes.bfloat16)
    DR = np.ascontiguousarray(
        np.asarray(D, dtype=np.float32).transpose(2, 0, 1).reshape(R, P * Q))
    BI = np.ascontiguousarray(np.asarray(bias, dtype=np.float32))

    in_maps = []
    for c in range(N_CORES):
        xtc = np.ascontiguousarray(
            x2[c * N_CORE:(c + 1) * N_CORE].T).astype(ml_dtypes.bfloat16)
        in_maps.append({
            "xt": xtc, "ct": CT, "bt": BT, "dr": DR, "biasd": BI,
        })
    return in_maps


def _run(in_maps, trace=False):
    global _cached_nc
    if _cached_nc is None:
        _cached_nc = _build()
    import time
    for attempt in range(3):
        try:
            return run_bass_kernel_spmd(
                _cached_nc, in_maps, list(range(N_CORES)), trace=trace)
        except Exception:
            # transient device errors (e.g. NRT_EXEC_UNIT_UNRECOVERABLE
            # from a previously wedged core) usually clear on retry
            if attempt == 2:
                raise
            time.sleep(5.0 * (attempt + 1))


def kernel(x, B, C, D, bias):
    lead = np.asarray(x).shape[:-1]
    res = _run(_prep_in_maps(x, B, C, D, bias))
    outs = [
        np.asarray(res.results[c]["outT"]).astype(np.float32).T
        for c in range(N_CORES)
    ]
    return np.concatenate(outs, axis=0).reshape(*lead, OUT_F)



# revision 29
# speedup vs baseline: 1.1639x; 1.1639x over previous
"""Trainium2 Bass kernel for nn_BlastLinear (block low-rank linear layer).

Math (reference):
  y[q,n,r] = sum_c x[n, q*1024+c] * C[q,r,c]          (mm1, per input block q)
  z[p,n,r] = sum_q D[p,q,r] * y[q,n,r]                (tiny mix over q)
  o[p,n,j] = sum_r z[p,n,r] * B[p,j,r]                (mm2, per output block p)
  out[n, p*1024+j] = o[p,n,j] + bias[p*1024+j]

Sharding: pure data-parallel over the 8192 tokens -> 1024 tokens per core,
weights replicated, no collectives.

Precision/speed: both matmuls run as fp8(e4m3) DoubleRow matmuls on hi/lo
SPLIT operands. Each logical bf16 k-tile product (a+b)(c+d) is computed as
the three fp8 products HH+HL+LH (the lo*lo term is ~eps^2 and dropped),
and DoubleRow packs TWO fp8 k-tile products per PE instruction at 0.5
cycles/row -> 1.5 instructions per logical k-tile = 0.75x the bf16 PE
time with ~bf16 accuracy (measured rel err ~3.3e-3 vs the 2e-2 gate;
single non-split e4m3 anywhere measures ~2.3e-2 and fails).
e4m3's narrow range forces power-of-2 pre-scaling: C,B ship as x64,
the mix uses D/2 so z' = 32z (~N(0,0.8), clear of e4m3 subnormals),
and PSUM o = 2048*o drains raw to bf16; the host applies /2048 + bias.

Per-core structure (chunk = 512 tokens, 2 chunks, PE order
mm1(c0), mm1(c1), mm2(c0), mm2(c1)):
  mm1:  psum y'[q,rt] [128r x 512n] += 12 DoubleRow matmuls per (q,rt)
        (classes HH/HL/LH over 4 k-tile pairs), x_hi/x_lo + C_hi/C_lo
        planes prepped on host.
  ycp:  yb = bf16(y') PSUM->SBUF on ACT
  mix:  zb[p,rt] += (D/2)[p,q,rt]*yb[q,rt]: mul (DVE 4x bf16) + adds
        (q1,q3 on DVE 2x; q2 on Pool)
  zsplit: zh = e4m3(zb) on ACT; zl = e4m3(zb - zh) (DVE on chunk 0,
        Pool on later chunks - keeps DVE ahead of mm2's deadline)
  mm2:  psum o[g] [128o x 512n] += 6 DoubleRow matmuls per (p,ot)
        (BH*zh, BL*zh, BH*zl over 2 rt-pairs)
  drain: raw copy PSUM->SBUF bf16 rotated over ACT/DVE/Pool; out ships
        bf16 TRANSPOSED [OUT_F, n_core] = 2048*o; host scales + bias.
DMA notes: ~25 MiB/core at ~360 GB/s effective sits under the 82us PE
floor; x ships as two fp8 planes (same bytes as bf16), C/B as hi/lo fp8
planes (same bytes as bf16). Batched transfers as in the bf16 baseline;
B prefetches during the last mm1 chunk; tail out-DMAs split small.
"""

import numpy as np

import concourse.mybir as mybir
import concourse.tile as tile
from concourse import bacc
from concourse.bass_utils import run_bass_kernel_spmd

N_CORES = 8
IN_F = 4096
OUT_F = 4096
P = 4
Q = 4
R = 512
CB = IN_F // Q        # 1024 input features per q block
OB = OUT_F // P       # 1024 output features per p block
N_TOK = 4 * 2048      # 8192 total tokens
N_CORE = N_TOK // N_CORES   # 1024 tokens per core

CHUNK = 512           # tokens per pipeline chunk
KT1 = CB // 128       # 8 contraction tiles per q in mm1
RT = R // 128         # 4 rank partition tiles
KB = 4                # k-tiles per x DMA batch
OG = 4                # o-groups per out DMA batch
OT = OB // 128        # 8 output-feature tiles per p

W_SCALE = 64.0        # C,B host pre-scale (keeps e4m3 operands normal)
Z_SCALE = 32.0        # z' = 32*z via D' = D*(Z_SCALE/W_SCALE)
O_SCALE = W_SCALE * Z_SCALE  # psum o = 2048*o; host divides

F32 = mybir.dt.float32
BF16 = mybir.dt.bfloat16
FP8 = mybir.dt.float8e4
ADD = mybir.AluOpType.add
SUB = mybir.AluOpType.subtract
DR = mybir.MatmulPerfMode.DoubleRow

_cached_nc = None


def _build(n_core=N_CORE, chunk=CHUNK):
    nc = bacc.Bacc("TRN2", target_bir_lowering=False, debug=False,
                   enable_asserts=False)

    def din(name, shape, dtype):
        return nc.dram_tensor(name, shape, dtype, kind="ExternalInput").ap()

    xh = din("xh", [IN_F, n_core], FP8)
    xl = din("xl", [IN_F, n_core], FP8)
    cth = din("cth", [IN_F, R], FP8)
    ctl = din("ctl", [IN_F, R], FP8)
    bth = din("bth", [P * R, OB], FP8)
    btl = din("btl", [P * R, OB], FP8)
    dr = din("dr", [R, P * Q], F32)
    outT = nc.dram_tensor("outT", [OUT_F, n_core], BF16,
                          kind="ExternalOutput").ap()

    n_chunks = n_core // chunk

    with tile.TileContext(nc) as tc:
        with (
            tc.tile_pool(name="const", bufs=1) as cpool,
            tc.tile_pool(name="xp", bufs=4) as xpool,
            tc.tile_pool(name="ybp", bufs=8) as ybpool,
            tc.tile_pool(name="tp", bufs=3) as tpool,
            tc.tile_pool(name="zbp", bufs=3 * n_chunks + 1) as zbpool,
            tc.tile_pool(name="zhp", bufs=2 * P * n_chunks) as zhpool,
            tc.tile_pool(name="outp", bufs=3) as outpool,
            tc.tile_pool(name="yps", bufs=2, space="PSUM") as ypool,
            tc.tile_pool(name="ops", bufs=2, space="PSUM") as opool,
        ):
            # C^T hi/lo planes: rows c = (q*8+k)*128 + c_
            cth_sb = cpool.tile([128, IN_F // 128, R], FP8)
            ctl_sb = cpool.tile([128, IN_F // 128, R], FP8)
            cth3 = cth.rearrange("(t p) r -> p t r", p=128)
            ctl3 = ctl.rearrange("(t p) r -> p t r", p=128)
            # B^T hi/lo planes: rows r = (p*4+rt)*128 + r_
            bth_sb = cpool.tile([128, (P * R) // 128, OB], FP8)
            btl_sb = cpool.tile([128, (P * R) // 128, OB], FP8)
            bth3 = bth.rearrange("(t p) o -> p t o", p=128)
            btl3 = btl.rearrange("(t p) o -> p t o", p=128)
            # d_sb[r_, rt, p*4+q] = (D/2)[p, q, rt*128 + r_]
            d_sb = cpool.tile([128, RT, P * Q], F32)

            zb = {}   # (j, p) -> z' bf16 plane [128, RT, chunk]
            zh = {}   # (j, p) -> [128, RT, chunk] fp8 hi plane
            zl = {}   # (j, p) -> [128, RT, chunk] fp8 lo plane

            def emit_mm1(j):
                last_c = j == n_chunks - 1
                for q in range(Q):
                    # two 2-bank PSUM pair tiles per q (rt 0,1 / 2,3)
                    ysp = [
                        ypool.tile([128, 2, chunk], F32, tag="y",
                                   name=f"y_{j}_{q}_{h}")
                        for h in range(RT // 2)
                    ]
                    for kb in range(KT1 // KB):
                        base_t = q * KT1 + kb * KB
                        first = j == 0 and q == 0 and kb == 0
                        xh_t = xpool.tile([128, KB, chunk], FP8, tag="xh",
                                          name=f"xh_{j}_{q}_{kb}")
                        xl_t = xpool.tile([128, KB, chunk], FP8, tag="xl",
                                          name=f"xl_{j}_{q}_{kb}")

                        def xdma(t, src, lo, hi):
                            nc.sync.dma_start(
                                t[:, lo:hi, :],
                                src[(base_t + lo) * 128:(base_t + hi) * 128,
                                    j * chunk:(j + 1) * chunk]
                                .rearrange("(t p) n -> p t n", p=128))

                        def cdma(dst, src, lo, hi):
                            hs = slice(base_t + lo, base_t + hi)
                            nc.sync.dma_start(dst[:, hs, :], src[:, hs, :])

                        if first:
                            # cold start: first k-pair's hi pieces ship
                            # alone so the first matmul waits on ~320KiB,
                            # then lo planes + rest stream under compute
                            cdma(cth_sb, cth3, 0, 2)
                            xdma(xh_t, xh, 0, 2)
                            cdma(ctl_sb, ctl3, 0, 2)
                            xdma(xl_t, xl, 0, 2)
                            cdma(cth_sb, cth3, 2, KB)
                            xdma(xh_t, xh, 2, KB)
                            cdma(ctl_sb, ctl3, 2, KB)
                            xdma(xl_t, xl, 2, KB)
                        else:
                            if j == 0:
                                cdma(cth_sb, cth3, 0, KB)
                            xdma(xh_t, xh, 0, KB)
                            if j == 0:
                                cdma(ctl_sb, ctl3, 0, KB)
                            xdma(xl_t, xl, 0, KB)
                        if j == 0 and q == 0 and kb == 1:
                            nc.sync.dma_start(
                                d_sb[:],
                                dr.rearrange("(t p) s -> p t s", p=128))
                        if j == n_chunks - 1:
                            # B stays out of the x stream except p0's
                            # warmup pieces in the last two slots; the
                            # rest ships during mm2(c0). n_chunks==1
                            # loads everything here.
                            idx = q * 2 + kb
                            if n_chunks == 1:
                                bs = slice(idx * 2, idx * 2 + 2)
                                nc.sync.dma_start(bth_sb[:, bs, :],
                                                  bth3[:, bs, :])
                                nc.sync.dma_start(btl_sb[:, bs, :],
                                                  btl3[:, bs, :])
                            elif idx == 6:
                                nc.sync.dma_start(bth_sb[:, 0:2, :],
                                                  bth3[:, 0:2, :])
                            elif idx == 7:
                                nc.sync.dma_start(btl_sb[:, 0:2, :],
                                                  btl3[:, 0:2, :])
                        # 3 fp8 product classes x 2 k-pairs x 4 rt tiles;
                        # class-major so the xl tiles are needed last
                        n_batches = KT1 // KB
                        for ci, (wsb, mt) in enumerate(
                                ((cth_sb, xh_t), (ctl_sb, xh_t),
                                 (cth_sb, xl_t))):
                            for kp in range(KB // 2):
                                for rt in range(RT):
                                    ks = slice(base_t + 2 * kp,
                                               base_t + 2 * kp + 2)
                                    ms = slice(2 * kp, 2 * kp + 2)
                                    nc.tensor.matmul(
                                        ysp[rt // 2][:, rt % 2, :],
                                        lhsT=wsb[:, ks,
                                                 rt * 128:(rt + 1) * 128],
                                        rhs=mt[:, ms, :],
                                        start=(kb == 0 and ci == 0
                                               and kp == 0),
                                        stop=(kb == n_batches - 1 and ci == 2
                                              and kp == KB // 2 - 1),
                                        perf_mode=DR)
                    # y' -> SBUF bf16 on ACT: one fused 2-bank drain per
                    # PSUM pair (1038ns vs 2x612)
                    ybp = []
                    for h in range(RT // 2):
                        yb_t = ybpool.tile([128, 2, chunk], BF16, tag="yb",
                                           name=f"yb_{j}_{q}_{h}")
                        nc.scalar.copy(yb_t[:], ysp[h][:])
                        ybp.append(yb_t)

                    def yb_ap(rt):
                        return ybp[rt // 2][:, rt % 2, :]

                    # mix entirely on DVE (mul 4x bf16, adds 2x bf16):
                    # Pool's slow TT ops stay OUT of the q1->q2->q3 chain.
                    # p-major so p0's z plane completes first and mm2 can
                    # start on it while later p's still mix.
                    for p in range(P):
                        if q == 0 and (j, p) not in zb:
                            zb[(j, p)] = zbpool.tile(
                                [128, RT, chunk], BF16, tag="zb",
                                name=f"zb_{j}_{p}")
                        zp = zb[(j, p)]
                        for rt in range(RT):
                            col = p * Q + q
                            dcol = d_sb[:, rt, col:col + 1]
                            if q == 0:
                                nc.vector.tensor_scalar_mul(
                                    zp[:, rt, :], yb_ap(rt), dcol)
                            else:
                                tt = tpool.tile([128, chunk], BF16, tag="t",
                                                name=f"t_{j}_{q}_{p}_{rt}")
                                nc.vector.tensor_scalar_mul(
                                    tt[:], yb_ap(rt), dcol)
                                nc.vector.tensor_tensor(
                                    zp[:, rt, :], tt[:], zp[:, rt, :],
                                    op=ADD)
                        if q == Q - 1:
                            # whole-plane hi/lo split ([128, RT*chunk]
                            # ops), both on Pool: any split on ACT head-
                            # of-line blocks later y/out drains behind
                            # its late-arriving deps (the q3 adds), which
                            # cascades; Pool has nothing else scheduled.
                            zh[(j, p)] = zhpool.tile(
                                [128, RT, chunk], FP8, tag="zh",
                                name=f"zh_{j}_{p}")
                            zl[(j, p)] = zhpool.tile(
                                [128, RT, chunk], FP8, tag="zl",
                                name=f"zl_{j}_{p}")
                            nc.gpsimd.tensor_copy(zh[(j, p)][:], zp[:])
                            nc.gpsimd.tensor_tensor(
                                zl[(j, p)][:], zp[:], zh[(j, p)][:], op=SUB)

            def emit_mm2(j):
                ob_t = None
                osp = None
                last = j == n_chunks - 1
                NG = P * OT
                if j == 0 and n_chunks > 1:
                    # finish B(p0) right before mm2 starts consuming it
                    nc.sync.dma_start(bth_sb[:, 2:4, :], bth3[:, 2:4, :])
                    nc.sync.dma_start(btl_sb[:, 2:4, :], btl3[:, 2:4, :])
                for p in range(P):
                    for ot in range(OT):
                        g = p * OT + ot
                        if j == 0 and n_chunks > 1 and p < P - 1 and ot < 2:
                            # stream B(p+1) hi/lo while mm2 runs on p
                            bs = slice((p + 1) * RT + 2 * ot,
                                       (p + 1) * RT + 2 * ot + 2)
                            nc.sync.dma_start(bth_sb[:, bs, :], bth3[:, bs, :])
                            nc.sync.dma_start(btl_sb[:, bs, :], btl3[:, bs, :])
                        if g % 2 == 0:
                            osp = opool.tile([128, 2, chunk], F32, tag="o",
                                             name=f"o_{j}_{g}")
                        ops = osp[:, g % 2, :]
                        # 3 fp8 classes x 2 rt-pairs of DoubleRow matmuls
                        n_i = 0
                        for wsb, mt in ((bth_sb, zh[(j, p)]),
                                        (btl_sb, zh[(j, p)]),
                                        (bth_sb, zl[(j, p)])):
                            for rp in range(RT // 2):
                                rs = slice(p * RT + 2 * rp,
                                           p * RT + 2 * rp + 2)
                                ms = slice(2 * rp, 2 * rp + 2)
                                nc.tensor.matmul(
                                    ops,
                                    lhsT=wsb[:, rs,
                                             ot * 128:(ot + 1) * 128],
                                    rhs=mt[:, ms, :],
                                    start=(n_i == 0), stop=(n_i == 5),
                                    perf_mode=DR)
                                n_i += 1
                        if g % OG == 0:
                            ob_t = outpool.tile([128, OG, chunk], BF16,
                                                tag="ob", name=f"ob_{j}_{g}")
                        if g % 2 == 0:
                            continue
                        # drain one 2-bank PSUM pair (groups g-1, g) in a
                        # single fused op; raw bf16 copies, scale/bias on
                        # host. Early chunks all on ACT; last chunk
                        # rotates ACT/DVE/Pool.
                        s0 = (g % OG) - 1
                        dst = ob_t[:, s0:s0 + 2, :]
                        if last and g >= NG - 2:
                            # final pair: halves on ACT and DVE in
                            # parallel, each group its own DMA
                            nc.scalar.copy(ob_t[:, s0, :], osp[:, 0, :])
                            nc.vector.tensor_copy(ob_t[:, s0 + 1, :],
                                                  osp[:, 1, :])
                            for gg in (g - 1, g):
                                nc.sync.dma_start(
                                    outT[gg * 128:(gg + 1) * 128,
                                         j * chunk:(j + 1) * chunk],
                                    ob_t[:, s0 + gg - g + 1, :])
                            continue
                        # Pool runs the z-splits. DVE still owns the
                        # later chunks' mix during mm2(c0), so early
                        # chunks drain on ACT alone; the last chunk
                        # alternates ACT/DVE.
                        if j < n_chunks - 1:
                            eng = nc.scalar
                        else:
                            eng = (nc.scalar, nc.vector)[(g // 2) % 2]
                        if eng is nc.scalar:
                            nc.scalar.copy(dst, osp[:])
                        else:
                            eng.tensor_copy(dst, osp[:])
                        if last and g == NG - 3:
                            # penultimate pair ships immediately as its
                            # own DMA so the tail stays short
                            nc.sync.dma_start(
                                outT[(g - 1) * 128:(g + 1) * 128,
                                     j * chunk:(j + 1) * chunk]
                                .rearrange("(t p) n -> p t n", p=128),
                                dst)
                        elif g % OG == OG - 1:
                            nc.sync.dma_start(
                                outT[(g - OG + 1) * 128:(g + 1) * 128,
                                     j * chunk:(j + 1) * chunk]
                                .rearrange("(t p) n -> p t n", p=128),
                                ob_t[:])

            for j in range(n_chunks):
                emit_mm1(j)
            for j in range(n_chunks):
                emit_mm2(j)

    nc.compile()
    return nc


def _split8(a):
    import ml_dtypes
    e4 = ml_dtypes.float8_e4m3
    hi = np.ascontiguousarray(a).astype(e4)
    lo = np.ascontiguousarray(a - hi.astype(np.float32)).astype(e4)
    return hi, lo


def _prep_in_maps(x, B, C, D, bias):
    x2 = np.asarray(x, dtype=np.float32).reshape(N_TOK, IN_F)
    CT = np.ascontiguousarray(
        (np.asarray(C, dtype=np.float32) * W_SCALE)
        .transpose(0, 2, 1).reshape(IN_F, R))
    cth, ctl = _split8(CT)
    BT = np.ascontiguousarray(
        (np.asarray(B, dtype=np.float32) * W_SCALE)
        .transpose(0, 2, 1).reshape(P * R, OB))
    bth, btl = _split8(BT)
    DRm = np.ascontiguousarray(
        (np.asarray(D, dtype=np.float32) * (Z_SCALE / W_SCALE))
        .transpose(2, 0, 1).reshape(R, P * Q))

    in_maps = []
    for c in range(N_CORES):
        xt = np.ascontiguousarray(x2[c * N_CORE:(c + 1) * N_CORE].T)
        xh_c, xl_c = _split8(xt)
        in_maps.append({
            "xh": xh_c, "xl": xl_c, "cth": cth, "ctl": ctl,
            "bth": bth, "btl": btl, "dr": DRm,
        })
    return in_maps


def _run(in_maps, trace=False):
    global _cached_nc
    if _cached_nc is None:
        _cached_nc = _build()
    import time
    for attempt in range(3):
        try:
            return run_bass_kernel_spmd(
                _cached_nc, in_maps, list(range(N_CORES)), trace=trace)
        except Exception:
            # transient device errors (e.g. NRT_EXEC_UNIT_UNRECOVERABLE
            # from a previously wedged core) usually clear on retry
            if attempt == 2:
                raise
            time.sleep(5.0 * (attempt + 1))


def kernel(x, B, C, D, bias):
    xa = np.asarray(x)
    lead = xa.shape[:-1]
    biasf = np.asarray(bias, dtype=np.float32)
    res = _run(_prep_in_maps(x, B, C, D, bias))
    outs = [
        np.asarray(res.results[c]["outT"]).astype(np.float32).T
        * (1.0 / O_SCALE) + biasf
        for c in range(N_CORES)
    ]
    return np.concatenate(outs, axis=0).reshape(*lead, OUT_F)
